# revision 6
# baseline (speedup 1.0000x reference)
"""GuidedResampler Trainium2 kernel.

Math reduction: in the reference, every high-res query q inside a 4x4 cell
maps to the same low-res row l = (h//4)*32 + (w//4), hence the same top-2
keys, the same softmax weights and the same gathered index set.  The output
is therefore constant within each 4x4 cell:

    P[c, cell]   = sum over the 4x4 patch of v[c, patch(cell)]      (sum-pool)
    (i1, i2)     = top-2 of coarse[l, :],  (w1, w2) = softmax(v1, v2)
    out_low[c,l] = (w1 * P[c, i1] + w2 * P[c, i2]) / 16
    out[c, h, w] = out_low[c, (h//4)*32 + w//4]                     (4x upsample)

Sharding: 8 cores = batch (4) x low-res row half (2), pure data parallel.
Each core gets the full v[b] (pool indices can point anywhere) and its half
of the coarse map rows, and produces out[b, :, 64*lh:64*lh+64, :].

On-core pipeline (single SPMD program, no partition-id dependence):
  - DMA coarse half  -> top-8 via DVE max / max_index -> (i1, i2, w1/16, w2/16)
    per 128-row tile, packed into Q[:, 0:4] columns.
  - Q transposed via PE, replicated across partitions with a K=1 ones-matmul
    -> i1_rep/i2_rep/w1_rep/w2_rep [128, 512].
  - DMA v in 4 chunks, 4x4 sum-pool via strided tensor_adds -> P [128, 1024],
    PE-transpose -> P^T tiles [128 cells, 128 C].
  - One-hot matrices G_k[key, l] = (i_k_rep - 128*kt == key_row) built with a
    single dual-op tensor_scalar per tile; A_k = P^T.T @ G_k accumulated on PE.
  - out_low = A1*w1_rep + A2*w2_rep, width-replicated x4 in SBUF, then 4
    row-strided DMAs write the x4 row-replicated output.
"""

import numpy as np

B, C, H, W = 4, 128, 128, 128
HL, WL = H // 4, W // 4          # 32 x 32 low-res grid
NL = HL * WL                     # 1024 low-res cells
LHALF = NL // 2                  # 512 coarse rows per core
N_CORES = 8

_CACHE = {}


def _emit(tc, nc, out_d, v_d, co_d, ctx):
    import concourse.mybir as mybir

    f32 = mybir.dt.float32
    i32 = mybir.dt.int32
    u32 = mybir.dt.uint32
    Alu = mybir.AluOpType
    Act = mybir.ActivationFunctionType

    pool_ = lambda **kw: ctx.enter_context(tc.tile_pool(**kw))
    consts = pool_(name="consts", bufs=1)
    inpool = pool_(name="inpool", bufs=1)
    vpool = pool_(name="vpool", bufs=2)
    ppool = pool_(name="ppool", bufs=2)
    small = pool_(name="small", bufs=4)
    gpool = pool_(name="gpool", bufs=1)
    cpool = pool_(name="cpool", bufs=2)
    wpool = pool_(name="wpool", bufs=2)
    psq = pool_(name="psq", bufs=1, space="PSUM")
    psrep = pool_(name="psrep", bufs=1, space="PSUM")
    pst = pool_(name="pst", bufs=1, space="PSUM")
    psa = pool_(name="psa", bufs=1, space="PSUM")

    # ---- constants -------------------------------------------------------
    ident = consts.tile([128, 128], f32, tag="ident")
    nc.gpsimd.memset(ident, 1.0)
    nc.gpsimd.affine_select(
        ident, ident, pattern=[[1, 128]], compare_op=Alu.is_equal,
        fill=0.0, base=0, channel_multiplier=-1,
    )
    keyi = consts.tile([128, 1], i32, tag="keyi")
    nc.gpsimd.iota(keyi, [[0, 1]], base=0, channel_multiplier=1)
    keyf = consts.tile([128, 1], f32, tag="keyf")
    nc.vector.tensor_copy(keyf, keyi)
    ones_row = consts.tile([1, 128], f32, tag="ones_row")
    nc.gpsimd.memset(ones_row, 1.0)

    # ---- coarse path: top-2 + softmax ------------------------------------
    co_sb = inpool.tile([128, 4, 1024], f32, tag="co")
    nc.sync.dma_start(out=co_sb, in_=co_d.rearrange("(t p) n -> p t n", p=128))

    rep_ps = [
        psrep.tile([128, LHALF], f32, tag=f"rep{c}", name=f"rep{c}")
        for c in range(4)
    ]
    for t in range(4):
        vals8 = small.tile([128, 8], f32, tag="vals8")
        inds8 = small.tile([128, 8], u32, tag="inds8")
        nc.vector.max(out=vals8, in_=co_sb[:, t, :])
        nc.vector.max_index(out=inds8, in_max=vals8, in_values=co_sb[:, t, :])

        q = small.tile([128, 4], f32, tag="q")
        nc.vector.tensor_copy(q[:, 0:2], inds8[:, 0:2])
        d = small.tile([128, 1], f32, tag="d")
        nc.vector.tensor_sub(d, vals8[:, 1:2], vals8[:, 0:1])  # v2 - v1
        # w1/16 = sigmoid(v1 - v2) / 16 ; w2/16 = 1/16 - w1/16
        nc.scalar.activation(out=q[:, 2:3], in_=d, func=Act.Sigmoid, scale=-1.0)
        nc.vector.tensor_scalar(q[:, 2:3], q[:, 2:3], 0.0625, None, op0=Alu.mult)
        nc.vector.tensor_scalar(
            q[:, 3:4], q[:, 2:3], -1.0, 0.0625, op0=Alu.mult, op1=Alu.add
        )

        for c in range(4):
            qt = psq.tile([1, 128], f32, tag="qt", name="qt")
            nc.tensor.transpose(qt, q[:, c:c + 1], ident)
            qr = small.tile([1, 128], f32, tag="qr", name="qr")
            nc.scalar.copy(out=qr, in_=qt)
            nc.tensor.matmul(
                rep_ps[c][:, 128 * t:128 * (t + 1)],
                ones_row, qr, start=True, stop=True,
            )

    i1r = consts.tile([128, LHALF], f32, tag="i1r")
    i2r = consts.tile([128, LHALF], f32, tag="i2r")
    w1r = consts.tile([128, LHALF], f32, tag="w1r")
    w2r = consts.tile([128, LHALF], f32, tag="w2r")
    for c, dst in enumerate((i1r, i2r, w1r, w2r)):
        nc.scalar.copy(out=dst, in_=rep_ps[c])

    # one-hot gather matrices, split DVE / GPSIMD
    g1s, g2s = [], []
    for kt in range(8):
        g1 = gpool.tile([128, LHALF], f32, tag=f"g1_{kt}")
        g2 = gpool.tile([128, LHALF], f32, tag=f"g2_{kt}")
        nc.vector.tensor_scalar(
            g1, i1r, float(128 * kt), keyf, op0=Alu.subtract, op1=Alu.is_equal
        )
        nc.gpsimd.tensor_scalar(
            g2, i2r, float(128 * kt), keyf, op0=Alu.subtract, op1=Alu.is_equal
        )
        g1s.append(g1)
        g2s.append(g2)

    # ---- v path: 4x4 sum-pool -> P, P^T ----------------------------------
    pacc = consts.tile([128, NL], f32, tag="P")
    pts = []
    for ch in range(4):
        vch = vpool.tile([128, 32, 128], f32, tag="vch")
        nc.sync.dma_start(out=vch, in_=v_d[:, 32 * ch:32 * (ch + 1), :])
        v4 = vch.rearrange("p h (w two) -> p h w two", two=2)
        s1 = ppool.tile([128, 32, 64], f32, tag="s1")
        nc.vector.tensor_add(s1, v4[:, :, :, 0], v4[:, :, :, 1])
        s14 = s1.rearrange("p h (w two) -> p h w two", two=2)
        s2 = ppool.tile([128, 32, 32], f32, tag="s2")
        nc.vector.tensor_add(s2, s14[:, :, :, 0], s14[:, :, :, 1])
        s24 = s2.rearrange("p (h two) w -> p h two w", two=2)
        s3 = ppool.tile([128, 16, 32], f32, tag="s3")
        nc.vector.tensor_add(s3, s24[:, :, 0, :], s24[:, :, 1, :])
        s34 = s3.rearrange("p (h two) w -> p h two w", two=2)
        pview = pacc[:, 256 * ch:256 * (ch + 1)].rearrange("p (h w) -> p h w", w=32)
        nc.vector.tensor_add(pview, s34[:, :, 0, :], s34[:, :, 1, :])

        for sub in range(2):
            t_idx = 2 * ch + sub
            ptp = pst.tile([128, 128], f32, tag="ptp")
            nc.tensor.transpose(ptp, pacc[:, 128 * t_idx:128 * (t_idx + 1)], ident)
            ptsb = gpool.tile([128, 128], f32, tag=f"pt_{t_idx}")
            nc.scalar.copy(out=ptsb, in_=ptp)
            pts.append(ptsb)

    # ---- gather matmuls + combine + upsample, in two l-halves ------------
    for hf in range(2):
        sl = slice(hf * 256, (hf + 1) * 256)
        a1 = psa.tile([128, 256], f32, tag="a1")
        a2 = psa.tile([128, 256], f32, tag="a2")
        for kt in range(8):
            nc.tensor.matmul(
                a1, pts[kt], g1s[kt][:, sl], start=(kt == 0), stop=(kt == 7)
            )
            nc.tensor.matmul(
                a2, pts[kt], g2s[kt][:, sl], start=(kt == 0), stop=(kt == 7)
            )
        t1 = cpool.tile([128, 256], f32, tag="t1")
        t2 = cpool.tile([128, 256], f32, tag="t2")
        nc.vector.tensor_mul(t1, a1, w1r[:, sl])
        nc.vector.tensor_mul(t2, a2, w2r[:, sl])
        nc.vector.tensor_add(t1, t1, t2)

        # width-replicate x4: [128, 8, 32] -> [128, 8, 128]
        ow = wpool.tile([128, 8, 128], f32, tag="ow")
        ow4 = ow.rearrange("p h (w f) -> p h w f", f=4)
        tlow = t1.rearrange("p (h w) -> p h w", w=32)
        for dc in range(4):
            eng = nc.vector if dc % 2 == 0 else nc.scalar
            if dc % 2 == 0:
                nc.vector.tensor_copy(ow4[:, :, :, dc], tlow)
            else:
                nc.scalar.copy(out=ow4[:, :, :, dc], in_=tlow)

        # row-replicate x4 via 4 strided DMAs; this half covers high rows
        # [32*hf, 32*hf+32)
        od4 = out_d[:, 32 * hf:32 * (hf + 1), :].rearrange(
            "c (i f) w -> c i f w", f=4
        )
        for dr in range(4):
            nc.sync.dma_start(out=od4[:, :, dr, :], in_=ow)


def _build():
    import concourse.bacc as bacc
    import concourse.mybir as mybir
    from concourse.tile import TileContext

    f32 = mybir.dt.float32
    nc = bacc.Bacc("TRN2", target_bir_lowering=False, debug=False,
                   num_devices=N_CORES)
    v_d = nc.dram_tensor("v", [C, H, W], f32, kind="ExternalInput")
    co_d = nc.dram_tensor("co", [LHALF, NL], f32, kind="ExternalInput")
    out_d = nc.dram_tensor("out", [C, H // 2, W], f32, kind="ExternalOutput")

    from contextlib import ExitStack

    with TileContext(nc) as tc, ExitStack() as ctx:
        _emit(tc, nc, out_d.ap(), v_d.ap(), co_d.ap(), ctx)
    nc.compile()
    return nc


def get_program():
    if "nc" not in _CACHE:
        _CACHE["nc"] = _build()
    return _CACHE["nc"]


def make_in_maps(v_high_feat, coarse_attn_map):
    in_maps = []
    for core in range(N_CORES):
        b, lh = core // 2, core % 2
        in_maps.append({
            "v": np.ascontiguousarray(v_high_feat[b]),
            "co": np.ascontiguousarray(
                coarse_attn_map[b, LHALF * lh:LHALF * (lh + 1)]
            ),
        })
    return in_maps


def assemble(results):
    out = np.empty((B, C, H, W), dtype=np.float32)
    for core in range(N_CORES):
        b, lh = core // 2, core % 2
        out[b, :, 64 * lh:64 * (lh + 1), :] = results[core]["out"]
    return out


def kernel(v_high_feat, coarse_attn_map):
    from concourse import bass_utils

    nc = get_program()
    in_maps = make_in_maps(
        np.asarray(v_high_feat, dtype=np.float32),
        np.asarray(coarse_attn_map, dtype=np.float32),
    )
    res = bass_utils.run_bass_kernel_spmd(
        nc, in_maps, core_ids=list(range(N_CORES))
    )
    return assemble(res.results)
